# revision 7
# baseline (speedup 1.0000x reference)
"""Additive (Bahdanau) attention on 8 TRN2 NeuronCores — self-contained Bass kernel.

Math: score(q,k) = w2 . tanh(hq[q] + hk[k] + b1) + b2;  out = softmax_k(score) @ V.

tanh(x) ~= a*x + c1*sin(w x) + c2*sin(2w x) + c4*sin(4w x)  (w = 0.64,
weighted LSQ under the empirical input law; e2e rel err ~6e-3 incl.
quantization).  Angle addition sin(w(a+b)) = sin(wa)cos(wb)+cos(wa)sin(wb)
turns the [B,Q,K,D] tanh+reduce into TensorE matmuls with contraction
(3 harmonics x 2 phases x D) = 1536.  Only ONE ACT sin/cos pair per side is
computed (|w h| <= 2.3 < pi; cos via sin(pi/2 - w|h|)); the 2w and 4w
harmonics come from double-angle algebra in fp16 (DVE 2x mode):
  p = s0 c0 (= sin2w/2), c1t = 1-2 s0^2 (= cos2w), e = p c1t (= sin4w/4),
  c4t = 1-8 p^2 (= cos4w);  s0^2 on ScalarE (Square shares the Sin table).

Structural points:
 - qT/kT come from hardware DMA-transpose loads (no PE transposes, no
   PSUM->SBUF copies).
 - h never lands in SBUF: Sin/|.| read the h PSUM banks directly; b1 enters
   the hk accumulation as a 1-row matmul (b1 x ones).
 - linear term a*x: the q-part cancels in softmax; the k-part is
   sum_e kT[e,k] * u_e with u = a * (Wk @ w2) precomputed on host, folded
   into the logits PSUM accumulation as two more matmuls.
 - w2/coef scales fold into per-partition scales of the F(query)-side tiles;
   softmax denominator = ones-column appended to V; 1/den folds into the
   final per-q scale done on ScalarE (Copy with per-partition scale).
 - activation tables are pre-warmed by dummy ops so Sin->Exp table switches
   hide under DMA/matmul phases.

Sharding: data-parallel over batch, B=16 -> 2 per core, no collectives.
"""

import math
from contextlib import ExitStack

import numpy as np
import ml_dtypes

import concourse.bass as bass
import concourse.mybir as mybir
import concourse.tile as tile
from concourse import bacc
from concourse.bass_utils import run_bass_kernel_spmd

F32 = mybir.dt.float32
BF16 = mybir.dt.bfloat16
FP16 = mybir.dt.float16
I32 = mybir.dt.int32
AF = mybir.ActivationFunctionType
ALU = mybir.AluOpType

NCORES = 8
B, NQ, NK, D = 16, 256, 256, 256
BL = B // NCORES          # local batches per core = 2
P = 128
DC = D // P               # d-chunks = 2
EC = D // P               # e-chunks = 2
QT = NQ // P
KT = NK // P
W = BL * NQ               # 512 cols per dt slice (b-major)
WF = DC * W               # 1024: F (or G) half width
HALFPI = math.pi / 2.0
ABS_MASK = 0x7FFFFFFF     # clears the fp32 sign bit

# tanh(x) ~= A*x + C1 sin(OM x) + C2 sin(2 OM x) + C4 sin(4 OM x)
OM = 0.64
A_LIN = 0.206043
C1 = 0.495931
C2 = 0.239591
C4 = 0.060320


def build_kernel() -> bacc.Bacc:
    nc = bacc.Bacc("TRN2", target_bir_lowering=False, debug=False)

    q_d = nc.dram_tensor("queries", [BL, NQ, D], BF16, kind="ExternalInput").ap()
    k_d = nc.dram_tensor("keys", [BL, NK, D], BF16, kind="ExternalInput").ap()
    v_d = nc.dram_tensor("values", [BL, NK, D], BF16, kind="ExternalInput").ap()
    wq_d = nc.dram_tensor("Wq", [D, D], BF16, kind="ExternalInput").ap()
    wk_d = nc.dram_tensor("Wk", [D, D], BF16, kind="ExternalInput").ap()
    # cb16: u_rep [*,0:512] | b1 row (row0, cols 512:768) | ones row (row0, 768:1280)
    cb_d = nc.dram_tensor("cb16", [P, 1280], BF16, kind="ExternalInput").ap()
    # cf32: C1w2(0,1) 2C2w2(2,3) 4C4w2(4,5) -32C4w2(6,7) halfpi(8)
    cf_d = nc.dram_tensor("cf32", [P, 9], F32, kind="ExternalInput").ap()
    out_d = nc.dram_tensor("out", [BL, NQ, D], F32, kind="ExternalOutput").ap()

    with tile.TileContext(nc) as tc, ExitStack() as ctx:
        cpool = ctx.enter_context(tc.tile_pool(name="consts", bufs=1))
        dpool = ctx.enter_context(tc.tile_pool(name="data", bufs=1))

        cf32 = cpool.tile([P, 9], F32)
        nc.sync.dma_start(cf32[:], cf_d[:])
        cb16 = cpool.tile([P, 1280], BF16)
        nc.sync.dma_start(cb16[:], cb_d[:])
        u_rep = cb16[:, 0:512]
        b1row = cb16[0:1, 512:768]
        onesrow = cb16[0:1, 768:1280]

        # transposed loads: qT/kT col = (ec*BL + b)*256 + x
        qTt = dpool.tile([P, EC * BL * NQ], BF16)
        kTt = dpool.tile([P, EC * BL * NK], BF16)
        for b in range(BL):
            nc.sync.dma_start_transpose(
                qTt[:].rearrange("p (ec b x) -> p ec b x", b=BL, x=NQ)[:, :, b, :],
                q_d[b])
        wq_sb = cpool.tile([P, EC * D], BF16)
        nc.sync.dma_start(wq_sb[:].rearrange("p (j e) -> p j e", e=D),
                          wq_d.rearrange("(j p) e -> p j e", p=P))
        for b in range(BL):
            nc.sync.dma_start_transpose(
                kTt[:].rearrange("p (ec b x) -> p ec b x", b=BL, x=NK)[:, :, b, :],
                k_d[b])
        wk_sb = cpool.tile([P, EC * D], BF16)
        nc.sync.dma_start(wk_sb[:].rearrange("p (j e) -> p j e", e=D),
                          wk_d.rearrange("(j p) e -> p j e", p=P))
        vb = dpool.tile([P, BL * KT * (D + 1)], BF16)  # 257-blocks: V | ones
        nc.sync.dma_start(
            vb[:].rearrange("p (b t c) -> p b t c", t=KT, c=D + 1)[:, :, :, 0:D],
            v_d.rearrange("b (t p) e -> p b t e", p=P))
        nc.gpsimd.memset(vb[:].rearrange("p (bt c) -> p bt c", c=D + 1)[:, :, D:D + 1], 1.0)

        # warm the trig table during the DMA phase
        scratch = cpool.tile([P, 2], F32)
        nc.scalar.activation(scratch[:, 0:1], cf32[:, 0:1], AF.Sin)

        # ---- h matmuls straight into wide PSUM (2 banks per side) ----
        hpool_cm = tc.tile_pool(name="hpsum", bufs=1, space="PSUM")
        hpool = hpool_cm.__enter__()
        h_f = hpool.tile([P, WF], F32, name="h_f", tag="h_f")
        h_g = hpool.tile([P, WF], F32, name="h_g", tag="h_g")
        for dt in range(DC):
            for ec in range(EC):
                nc.tensor.matmul(
                    h_f[:, dt * W:(dt + 1) * W],
                    wq_sb[:, ec * D + dt * P:ec * D + (dt + 1) * P],
                    qTt[:, ec * W:(ec + 1) * W],
                    start=(ec == 0), stop=(ec == EC - 1))
        for dt in range(DC):
            for ec in range(EC):
                nc.tensor.matmul(
                    h_g[:, dt * W:(dt + 1) * W],
                    wk_sb[:, ec * D + dt * P:ec * D + (dt + 1) * P],
                    kTt[:, ec * W:(ec + 1) * W],
                    start=(ec == 0), stop=False)
            # b1 enters as a rank-1 (1-row) matmul: b1[d] x ones[b*q]
            nc.tensor.matmul(
                h_g[:, dt * W:(dt + 1) * W],
                b1row[:, dt * P:(dt + 1) * P],
                onesrow[:],
                start=False, stop=True)

        # ---- activations + harmonic algebra (F cols 0:1024, G cols 1024:2048) ----
        s0 = dpool.tile([P, 2 * WF], FP16)
        c0 = dpool.tile([P, 2 * WF], FP16)
        sq = dpool.tile([P, 2 * WF], FP16)    # s0^2
        pp = dpool.tile([P, 2 * WF], FP16)    # s0*c0 = sin2w/2
        c1t = dpool.tile([P, 2 * WF], FP16)   # 1-2 s0^2 = cos2w
        ee = dpool.tile([P, 2 * WF], FP16)    # p*c1t = sin4w/4
        p2 = dpool.tile([P, 2 * WF], FP16)    # p^2
        c4g = dpool.tile([P, WF], FP16)       # G: 1-8 p^2 = cos4w
        habs = dpool.tile([P, 2 * WF], F32)   # |h|
        GO = WF

        nc.vector.tensor_scalar(habs[:, 0:WF].bitcast(I32), h_f[:].bitcast(I32),
                                ABS_MASK, None, op0=ALU.bitwise_and)
        nc.vector.tensor_scalar(habs[:, GO:].bitcast(I32), h_g[:].bitcast(I32),
                                ABS_MASK, None, op0=ALU.bitwise_and)
        nc.scalar.activation(s0[:, 0:WF], h_f[:], AF.Sin, bias=0.0, scale=OM)
        nc.scalar.activation(s0[:, GO:], h_g[:], AF.Sin, bias=0.0, scale=OM)
        hpool_cm.__exit__(None, None, None)   # release h PSUM banks for attnV
        nc.scalar.activation(c0[:], habs[:], AF.Sin, bias=cf32[:, 8:9], scale=-OM)
        nc.scalar.activation(sq[:], s0[:], AF.Square)

        nc.vector.tensor_tensor(pp[:], s0[:], c0[:], op=ALU.mult)
        nc.vector.tensor_scalar(c1t[:], sq[:], -2.0, 1.0, op0=ALU.mult, op1=ALU.add)
        nc.vector.tensor_tensor(ee[:], pp[:], c1t[:], op=ALU.mult)
        nc.vector.tensor_tensor(p2[:], pp[:], pp[:], op=ALU.mult)
        nc.gpsimd.tensor_scalar(c4g[:], p2[:, GO:], -8.0, 1.0, op0=ALU.mult, op1=ALU.add)

        # F-side tiles scaled by per-partition (w2 * coef); DVE for early, Pool late
        sF1 = dpool.tile([P, WF], FP16)
        cF1 = dpool.tile([P, WF], FP16)
        sF2 = dpool.tile([P, WF], FP16)
        cF2 = dpool.tile([P, WF], FP16)
        sF4 = dpool.tile([P, WF], FP16)
        cF4 = dpool.tile([P, WF], FP16)
        for dt in range(DC):
            sl = slice(dt * W, (dt + 1) * W)
            nc.vector.tensor_scalar_mul(sF1[:, sl], s0[:, sl], cf32[:, 0 + dt:1 + dt])
            nc.vector.tensor_scalar_mul(cF1[:, sl], c0[:, sl], cf32[:, 0 + dt:1 + dt])
            nc.vector.tensor_scalar_mul(sF2[:, sl], pp[:, sl], cf32[:, 2 + dt:3 + dt])
            nc.gpsimd.tensor_scalar_mul(cF2[:, sl], c1t[:, sl], cf32[:, 2 + dt:3 + dt])
            nc.gpsimd.tensor_scalar_mul(sF4[:, sl], ee[:, sl], cf32[:, 4 + dt:5 + dt])
            nc.gpsimd.tensor_scalar(cF4[:, sl], p2[:, sl], cf32[:, 6 + dt:7 + dt],
                                    cf32[:, 4 + dt:5 + dt], op0=ALU.mult, op1=ALU.add)

        # ---- logits accumulation: beta (linear term) + 3 harmonics ----
        wpool = ctx.enter_context(tc.tile_pool(name="wpsum", bufs=4, space="PSUM"))
        logits_ps = [[wpool.tile([P, NQ], F32, name=f"lg_{kt}_{b}", tag="work")
                      for b in range(BL)] for kt in range(KT)]

        # beta: logits^T[k, q] += sum_e kT[e, k] * u_e   (u = a * Wk @ w2)
        for kt in range(KT):
            for b in range(BL):
                for ec in range(EC):
                    nc.tensor.matmul(
                        logits_ps[kt][b][:],
                        kTt[:, ec * W + b * NQ + kt * P:ec * W + b * NQ + kt * P + P],
                        u_rep[:, ec * NQ:(ec + 1) * NQ],
                        start=(ec == 0), stop=False)
        # harmonic terms: (G raw, from col GO) x (F scaled)
        TERMS = ((c0, sF1), (s0, cF1), (c1t, sF2), (pp, cF2), (None, sF4), (ee, cF4))
        for ti, (gt, ft) in enumerate(TERMS):
            last = ti == len(TERMS) - 1
            for dt in range(DC):
                for b in range(BL):
                    for kt in range(KT):
                        o = dt * W + b * NQ + kt * P
                        g = c4g[:, o:o + P] if gt is None else gt[:, GO + o:GO + o + P]
                        nc.tensor.matmul(
                            logits_ps[kt][b][:],
                            g,
                            ft[:, dt * W + b * NQ:dt * W + (b + 1) * NQ],
                            start=False, stop=(last and dt == DC - 1))

        # warm the exp table while the last matmuls run
        nc.scalar.activation(scratch[:, 1:2], cf32[:, 0:1], AF.Exp)

        expT = dpool.tile([P, KT * BL * NQ], BF16)
        for kt in range(KT):
            for b in range(BL):
                nc.scalar.activation(
                    expT[:, (kt * BL + b) * NQ:(kt * BL + b + 1) * NQ],
                    logits_ps[kt][b][:], AF.Exp)

        # ---- attn @ [V|1]; 1/denominator folds into the output scale ----
        apool = ctx.enter_context(tc.tile_pool(name="apsum", bufs=4, space="PSUM"))
        recip_sb = cpool.tile([P, BL * QT], F32)
        out_sb = dpool.tile([P, BL * QT * D], F32)
        for qt in range(QT):
            for b in range(BL):
                av = apool.tile([P, D + 1], F32, name=f"av_{qt}_{b}", tag="av")
                for kt in range(KT):
                    nc.tensor.matmul(
                        av[:],
                        expT[:, (kt * BL + b) * NQ + qt * P:(kt * BL + b) * NQ + (qt + 1) * P],
                        vb[:, (b * KT + kt) * (D + 1):(b * KT + kt + 1) * (D + 1)],
                        start=(kt == 0), stop=(kt == KT - 1))
                i = b * QT + qt
                nc.vector.reciprocal(recip_sb[:, i:i + 1], av[:, D:D + 1])
                # per-q normalization on ScalarE: Copy with per-partition scale
                nc.scalar.activation(out_sb[:, i * D:(i + 1) * D], av[:, 0:D],
                                     AF.Copy, scale=recip_sb[:, i:i + 1])
            nc.gpsimd.dma_start(
                out_d.rearrange("b (t p) e -> p b t e", p=P)[:, :, qt, :],
                out_sb[:].rearrange("p (b t e) -> p b t e", t=QT, e=D)[:, :, qt, :])

    nc.compile()
    return nc


def _host_tables(b1: np.ndarray, w2: np.ndarray, Wk_bf: np.ndarray):
    cf32 = np.zeros((P, 9), np.float32)
    for dt in range(DC):
        wv = w2[dt * P:(dt + 1) * P]
        cf32[:, 0 + dt] = C1 * wv
        cf32[:, 2 + dt] = 2.0 * C2 * wv
        cf32[:, 4 + dt] = 4.0 * C4 * wv
        cf32[:, 6 + dt] = -32.0 * C4 * wv
    cf32[:, 8] = HALFPI
    u = A_LIN * (Wk_bf.astype(np.float64) @ w2)      # [256]
    cb16 = np.zeros((P, 1280), np.float32)
    for ec in range(EC):
        cb16[:, ec * NQ:(ec + 1) * NQ] = u[ec * P:(ec + 1) * P][:, None]
    cb16[0, 512:768] = b1
    cb16[0, 768:1280] = 1.0
    return cf32, np.ascontiguousarray(cb16.astype(ml_dtypes.bfloat16))


_NC_CACHE = {}


def _get_nc():
    if "nc" not in _NC_CACHE:
        _NC_CACHE["nc"] = build_kernel()
    return _NC_CACHE["nc"]


def _make_in_maps(inputs):
    keys = np.ascontiguousarray(np.asarray(inputs["keys"], np.float32).astype(ml_dtypes.bfloat16))
    queries = np.ascontiguousarray(np.asarray(inputs["queries"], np.float32).astype(ml_dtypes.bfloat16))
    values = np.ascontiguousarray(np.asarray(inputs["values"], np.float32).astype(ml_dtypes.bfloat16))
    Wk = np.ascontiguousarray(np.asarray(inputs["Wk"], np.float32).astype(ml_dtypes.bfloat16))
    Wq = np.ascontiguousarray(np.asarray(inputs["Wq"], np.float32).astype(ml_dtypes.bfloat16))
    b1 = np.asarray(inputs["b1"], np.float64)
    w2 = np.asarray(inputs["w2"], np.float64)
    cf32, cb16 = _host_tables(b1, w2, Wk)

    in_maps = []
    for c in range(NCORES):
        sl = slice(c * BL, (c + 1) * BL)
        in_maps.append({
            "queries": queries[sl], "keys": keys[sl], "values": values[sl],
            "Wq": Wq, "Wk": Wk, "cf32": cf32, "cb16": cb16,
        })
    return in_maps


def _run(inputs, trace=False, trace_kwargs=None):
    nc = _get_nc()
    in_maps = _make_in_maps(inputs)
    kwargs = {}
    if trace:
        kwargs = dict(trace=True, trace_cores=[0], trace_kwargs=trace_kwargs or {})
    res = run_bass_kernel_spmd(nc, in_maps, core_ids=list(range(NCORES)), **kwargs)
    out = np.concatenate([res.results[c]["out"] for c in range(NCORES)], axis=0)
    return out, res


def kernel(**inputs) -> np.ndarray:
    out, _ = _run(inputs, trace=False)
    return out


# revision 8
# speedup vs baseline: 1.6803x; 1.6803x over previous
"""Additive (Bahdanau) attention on 8 TRN2 NeuronCores — self-contained Bass kernel.

Math: score(q,k) = w2 . tanh(hq[q] + hk[k] + b1) + b2;  out = softmax_k(score) @ V.

tanh(x) ~= a*x + c1*sin(w x) + c2*sin(2w x) + c4*sin(4w x)  (w = 0.64,
weighted LSQ under the empirical input law; e2e rel err ~6e-3 incl.
quantization).  Angle addition turns the [B,Q,K,D] tanh+reduce into TensorE
matmuls with contraction (3 harmonics x 2 phases x D) = 1536.  Only ONE ACT
sin/cos pair per side is computed (|w h| <= 2.3 < pi; cos via
sin(pi/2 - w|h|)); the 2w / 4w harmonics come from double-angle algebra in
fp16 on the DVE (2x mode):  p = s0 c0, c2w = 1-2 s0^2, e = p c2w (= s4w/4),
c4w = 1-8 p^2;  s0^2 runs on ScalarE (Square shares the Sin table).

Structural points:
 - qT/kT via hardware DMA-transpose loads (no PE transposes / PSUM copies).
 - h never lands in SBUF: Sin and |.| read the h PSUM banks directly; b1
   enters the hk accumulation as a rank-1 (1-row) matmul b1 x ones.
 - linear term a*x: the q-part cancels in softmax; the k-part is
   sum_e kT[e,k] * u_e with u = a*(Wk @ w2) host-precomputed, folded into
   the logits PSUM accumulation.
 - w2*coef scales fold into the F(query)-side tiles via tensor_tensor
   against replicated fp16 constants (AP-scalar tensor_scalar ops hit an
   erratic 128x-slow per-partition path on HW — avoided).
 - softmax denominator = ones-column appended to V; 1/den folds into the
   final per-q scale done on ScalarE (Copy with per-partition scale).
 - logits / h / attn PSUM each live in one wide multi-bank tile so exp and
   reciprocal run as single strided ops.
 - input DMAs are spread across the SP/ACT/Pool queues (serial issuance on
   one queue was gating the whole pipeline); activation tables are
   pre-warmed by dummy ops ordered via data deps.

Sharding: data-parallel over batch, B=16 -> 2 per core, no collectives.
"""

import math

import numpy as np
import ml_dtypes

import concourse.bass as bass
import concourse.mybir as mybir
import concourse.tile as tile
from concourse import bacc
from concourse.bass_utils import run_bass_kernel_spmd

F32 = mybir.dt.float32
BF16 = mybir.dt.bfloat16
FP16 = mybir.dt.float16
I32 = mybir.dt.int32
AF = mybir.ActivationFunctionType
ALU = mybir.AluOpType

NCORES = 8
B, NQ, NK, D = 16, 256, 256, 256
BL = B // NCORES          # local batches per core = 2
P = 128
DC = D // P               # d-chunks = 2
EC = D // P               # e-chunks = 2
QT = NQ // P
KT = NK // P
W = BL * NQ               # 512 cols per dt slice (b-major)
WF = DC * W               # 1024: F (or G) half width
GO = WF                   # G half offset
HALFPI = math.pi / 2.0
ABS_MASK = 0x7FFFFFFF     # clears the fp32 sign bit

# tanh(x) ~= A*x + C1 sin(OM x) + C2 sin(2 OM x) + C4 sin(4 OM x)
OM = 0.64
A_LIN = 0.206043
C1 = 0.495931
C2 = 0.239591
C4 = 0.060320


def build_kernel() -> bacc.Bacc:
    nc = bacc.Bacc("TRN2", target_bir_lowering=False, debug=False)

    q_d = nc.dram_tensor("queries", [BL, NQ, D], BF16, kind="ExternalInput").ap()
    k_d = nc.dram_tensor("keys", [BL, NK, D], BF16, kind="ExternalInput").ap()
    v_d = nc.dram_tensor("values", [BL, NK, D], BF16, kind="ExternalInput").ap()
    wq_d = nc.dram_tensor("Wq", [D, D], BF16, kind="ExternalInput").ap()
    wk_d = nc.dram_tensor("Wk", [D, D], BF16, kind="ExternalInput").ap()
    # cb16: u_rep [*,0:512] | b1 row (row0, 512:768) | ones row (row0, 768:1280)
    cb_d = nc.dram_tensor("cb16", [P, 1280], BF16, kind="ExternalInput").ap()
    # cfp16: F-side scale tiles: C1w2 | 2C2w2 | 4C4w2, each [128, 1024]
    cp_d = nc.dram_tensor("cfp16", [P, 3 * WF], FP16, kind="ExternalInput").ap()
    # cf32: col0 = pi/2 (Sin bias + warm input)
    cf_d = nc.dram_tensor("cf32", [P, 1], F32, kind="ExternalInput").ap()
    out_d = nc.dram_tensor("out", [BL, NQ, D], F32, kind="ExternalOutput").ap()

    with tile.TileContext(nc) as tc:
        cpool_cm = tc.tile_pool(name="consts", bufs=1)
        cpool = cpool_cm.__enter__()
        dpool_cm = tc.tile_pool(name="data", bufs=1)
        dpool = dpool_cm.__enter__()

        # ---- constants + inputs, spread across queues ----
        cf32 = cpool.tile([P, 1], F32)
        nc.gpsimd.dma_start(cf32[:], cf_d[:])          # tiny, first: unblocks warm
        cb16 = cpool.tile([P, 1280], BF16)
        nc.gpsimd.dma_start(cb16[:], cb_d[:])
        u_rep = cb16[:, 0:512]
        b1row = cb16[0:1, 512:768]
        onesrow = cb16[0:1, 768:1280]
        reps = cpool.tile([P, 3 * WF], FP16)
        nc.gpsimd.dma_start(reps[:], cp_d[:])
        rep1 = reps[:, 0:WF]
        rep2 = reps[:, WF:2 * WF]
        rep4 = reps[:, 2 * WF:]
        vb = dpool.tile([P, BL * KT * (D + 1)], BF16)  # 257-blocks: V | ones
        nc.gpsimd.dma_start(
            vb[:].rearrange("p (b t c) -> p b t c", t=KT, c=D + 1)[:, :, :, 0:D],
            v_d.rearrange("b (t p) e -> p b t e", p=P))
        nc.gpsimd.memset(vb[:].rearrange("p (bt c) -> p bt c", c=D + 1)[:, :, D:D + 1], 1.0)

        # transposed loads: qT/kT col = (ec*BL + b)*256 + x
        qTt = dpool.tile([P, EC * BL * NQ], BF16)
        kTt = dpool.tile([P, EC * BL * NK], BF16)
        for b in range(BL):
            nc.sync.dma_start_transpose(
                qTt[:].rearrange("p (ec b x) -> p ec b x", b=BL, x=NQ)[:, :, b, :],
                q_d[b])
        wq_sb = cpool.tile([P, EC * D], BF16)
        nc.sync.dma_start(wq_sb[:].rearrange("p (j e) -> p j e", e=D),
                          wq_d.rearrange("(j p) e -> p j e", p=P))
        for b in range(BL):
            nc.scalar.dma_start_transpose(
                kTt[:].rearrange("p (ec b x) -> p ec b x", b=BL, x=NK)[:, :, b, :],
                k_d[b])
        wk_sb = cpool.tile([P, EC * D], BF16)
        nc.sync.dma_start(wk_sb[:].rearrange("p (j e) -> p j e", e=D),
                          wk_d.rearrange("(j p) e -> p j e", p=P))

        # warm the trig table during the DMA phase
        scratch = cpool.tile([P, 2], F32)
        nc.scalar.activation(scratch[:, 0:1], cf32[:, 0:1], AF.Sin)

        # ---- h matmuls into one wide PSUM tile (4 banks, 4 acc groups) ----
        # F dt-groups at cols dt*512; G at 1024 + dt*512
        hpool_cm = tc.tile_pool(name="hpsum", bufs=1, space="PSUM")
        hpool = hpool_cm.__enter__()
        h_fg = hpool.tile([P, 2 * WF], F32, name="h_fg", tag="h_fg")
        for dt in range(DC):
            for ec in range(EC):
                nc.tensor.matmul(
                    h_fg[:, dt * W:(dt + 1) * W],
                    wq_sb[:, ec * D + dt * P:ec * D + (dt + 1) * P],
                    qTt[:, ec * W:(ec + 1) * W],
                    start=(ec == 0), stop=(ec == EC - 1))
        for dt in range(DC):
            for ec in range(EC):
                nc.tensor.matmul(
                    h_fg[:, GO + dt * W:GO + (dt + 1) * W],
                    wk_sb[:, ec * D + dt * P:ec * D + (dt + 1) * P],
                    kTt[:, ec * W:(ec + 1) * W],
                    start=(ec == 0), stop=False)
            # b1 enters as a rank-1 (1-row) matmul: b1[d] x ones[b*q]
            nc.tensor.matmul(
                h_fg[:, GO + dt * W:GO + (dt + 1) * W],
                b1row[:, dt * P:(dt + 1) * P],
                onesrow[:],
                start=False, stop=True)

        # ---- activations + harmonic algebra (F cols 0:1024, G 1024:2048) ----
        s0 = dpool.tile([P, 2 * WF], FP16)
        c0 = dpool.tile([P, 2 * WF], FP16)
        sq = dpool.tile([P, 2 * WF], FP16)    # s0^2
        pp = dpool.tile([P, 2 * WF], FP16)    # s0*c0 = sin2w/2
        c1t = dpool.tile([P, 2 * WF], FP16)   # 1-2 s0^2 = cos2w
        ee = dpool.tile([P, 2 * WF], FP16)    # p*c1t = sin4w/4
        p2 = dpool.tile([P, 2 * WF], FP16)    # p^2
        c4t = dpool.tile([P, 2 * WF], FP16)   # 1-8 p^2 = cos4w
        habs = dpool.tile([P, 2 * WF], F32)   # |h|

        nc.vector.tensor_scalar(habs[:, 0:WF].bitcast(I32), h_fg[:, 0:WF].bitcast(I32),
                                ABS_MASK, None, op0=ALU.bitwise_and)
        nc.vector.tensor_scalar(habs[:, GO:].bitcast(I32), h_fg[:, GO:].bitcast(I32),
                                ABS_MASK, None, op0=ALU.bitwise_and)
        nc.scalar.activation(s0[:, 0:WF], h_fg[:, 0:WF], AF.Sin, bias=0.0, scale=OM)
        nc.scalar.activation(s0[:, GO:], h_fg[:, GO:], AF.Sin, bias=0.0, scale=OM)
        hpool_cm.__exit__(None, None, None)   # release h banks for attnV
        nc.scalar.activation(c0[:], habs[:], AF.Sin, bias=cf32[:, 0:1], scale=-OM)
        nc.scalar.activation(sq[:], s0[:], AF.Square)

        nc.vector.tensor_tensor(pp[:], s0[:], c0[:], op=ALU.mult)
        nc.vector.tensor_scalar(c1t[:], sq[:], -2.0, 1.0, op0=ALU.mult, op1=ALU.add)
        nc.vector.tensor_tensor(ee[:], pp[:], c1t[:], op=ALU.mult)
        nc.vector.tensor_tensor(p2[:], pp[:], pp[:], op=ALU.mult)
        nc.vector.tensor_scalar(c4t[:], p2[:], -8.0, 1.0, op0=ALU.mult, op1=ALU.add)

        # F-side tiles scaled by (w2*coef) via TT against replicated consts
        sF1 = dpool.tile([P, WF], FP16)
        cF1 = dpool.tile([P, WF], FP16)
        sF2 = dpool.tile([P, WF], FP16)
        cF2 = dpool.tile([P, WF], FP16)
        sF4 = dpool.tile([P, WF], FP16)
        cF4 = dpool.tile([P, WF], FP16)
        nc.vector.tensor_tensor(sF1[:], s0[:, 0:WF], rep1, op=ALU.mult)
        nc.vector.tensor_tensor(cF1[:], c0[:, 0:WF], rep1, op=ALU.mult)
        nc.vector.tensor_tensor(sF2[:], pp[:, 0:WF], rep2, op=ALU.mult)
        nc.vector.tensor_tensor(cF2[:], c1t[:, 0:WF], rep2, op=ALU.mult)
        nc.vector.tensor_tensor(sF4[:], ee[:, 0:WF], rep4, op=ALU.mult)
        nc.vector.tensor_tensor(cF4[:], c4t[:, 0:WF], rep4, op=ALU.mult)

        # ---- logits: one wide PSUM tile, group (kt,b) at col (kt*2+b)*512 ----
        wpool_cm = tc.tile_pool(name="wpsum", bufs=1, space="PSUM")
        wpool = wpool_cm.__enter__()
        lg = wpool.tile([P, 4 * 512], F32, name="lg", tag="lg")

        def lsl(kt, b):
            o = (kt * BL + b) * 512
            return lg[:, o:o + NQ]

        # beta: logits^T[k, q] += sum_e kT[e, k] * u_e   (u = a * Wk @ w2)
        for kt in range(KT):
            for b in range(BL):
                for ec in range(EC):
                    nc.tensor.matmul(
                        lsl(kt, b),
                        kTt[:, ec * W + b * NQ + kt * P:ec * W + b * NQ + kt * P + P],
                        u_rep[:, ec * NQ:(ec + 1) * NQ],
                        start=(ec == 0), stop=False)
        # harmonic terms: (G raw, col GO+) x (F scaled)
        TERMS = ((c0, sF1), (s0, cF1), (c1t, sF2), (pp, cF2), (c4t, sF4), (ee, cF4))
        for ti, (gt, ft) in enumerate(TERMS):
            last = ti == len(TERMS) - 1
            for dt in range(DC):
                for b in range(BL):
                    for kt in range(KT):
                        o = GO + dt * W + b * NQ + kt * P
                        nc.tensor.matmul(
                            lsl(kt, b),
                            gt[:, o:o + P],
                            ft[:, dt * W + b * NQ:dt * W + (b + 1) * NQ],
                            start=False, stop=(last and dt == DC - 1))

        # warm the exp table; input dep on sq pins it after the Square pass
        nc.scalar.activation(scratch[:, 1:2], sq[:, 0:1], AF.Exp)

        # exp of all four logit groups in one strided ACTIVATE
        expT = dpool.tile([P, KT * BL * NQ], BF16)
        nc.scalar.activation(
            expT[:].rearrange("p (g x) -> p g x", x=NQ),
            lg[:].rearrange("p (g x) -> p g x", x=512)[:, :, 0:NQ],
            AF.Exp)

        # ---- attn @ [V|1]; group (qt,b) at col (qt*2+b)*512, width 257 ----
        apool_cm = tc.tile_pool(name="apsum", bufs=1, space="PSUM")
        apool = apool_cm.__enter__()
        av = apool.tile([P, 4 * 512], F32, name="av", tag="av")
        recip_sb = cpool.tile([P, BL * QT], F32)
        out_sb = dpool.tile([P, BL * QT * D], F32)
        for qt in range(QT):
            for b in range(BL):
                o = (qt * BL + b) * 512
                for kt in range(KT):
                    nc.tensor.matmul(
                        av[:, o:o + D + 1],
                        expT[:, (kt * BL + b) * NQ + qt * P:(kt * BL + b) * NQ + (qt + 1) * P],
                        vb[:, (b * KT + kt) * (D + 1):(b * KT + kt + 1) * (D + 1)],
                        start=(kt == 0), stop=(kt == KT - 1))
            # both b of this qt: one strided reciprocal, two scaled copies
            nc.vector.reciprocal(
                recip_sb[:, qt * BL:(qt + 1) * BL],
                av[:].rearrange("p (g x) -> p g x", x=512)[:, qt * BL:(qt + 1) * BL, D:D + 1])
            for b in range(BL):
                i = qt * BL + b
                nc.scalar.activation(out_sb[:, (b * QT + qt) * D:(b * QT + qt + 1) * D],
                                     av[:, (qt * BL + b) * 512:(qt * BL + b) * 512 + D],
                                     AF.Copy, scale=recip_sb[:, i:i + 1])
            nc.gpsimd.dma_start(
                out_d.rearrange("b (t p) e -> p b t e", p=P)[:, :, qt, :],
                out_sb[:].rearrange("p (b t e) -> p b t e", t=QT, e=D)[:, :, qt, :])
        apool_cm.__exit__(None, None, None)
        wpool_cm.__exit__(None, None, None)
        dpool_cm.__exit__(None, None, None)
        cpool_cm.__exit__(None, None, None)

    nc.compile()
    return nc


def _host_tables(b1: np.ndarray, w2: np.ndarray, Wk_bf: np.ndarray):
    cf32 = np.full((P, 1), HALFPI, np.float32)
    u = A_LIN * (Wk_bf.astype(np.float64) @ w2)      # [256]
    cb16 = np.zeros((P, 1280), np.float32)
    for ec in range(EC):
        cb16[:, ec * NQ:(ec + 1) * NQ] = u[ec * P:(ec + 1) * P][:, None]
    cb16[0, 512:768] = b1
    cb16[0, 768:1280] = 1.0
    cfp16 = np.zeros((P, 3 * WF), np.float32)
    for dt in range(DC):
        wv = w2[dt * P:(dt + 1) * P]
        for mi, coef in enumerate((C1, 2.0 * C2, 4.0 * C4)):
            cfp16[:, mi * WF + dt * W:mi * WF + (dt + 1) * W] = (coef * wv)[:, None]
    return (cf32,
            np.ascontiguousarray(cb16.astype(ml_dtypes.bfloat16)),
            np.ascontiguousarray(cfp16.astype(np.float16)))


_NC_CACHE = {}


def _get_nc():
    if "nc" not in _NC_CACHE:
        _NC_CACHE["nc"] = build_kernel()
    return _NC_CACHE["nc"]


def _make_in_maps(inputs):
    keys = np.ascontiguousarray(np.asarray(inputs["keys"], np.float32).astype(ml_dtypes.bfloat16))
    queries = np.ascontiguousarray(np.asarray(inputs["queries"], np.float32).astype(ml_dtypes.bfloat16))
    values = np.ascontiguousarray(np.asarray(inputs["values"], np.float32).astype(ml_dtypes.bfloat16))
    Wk = np.ascontiguousarray(np.asarray(inputs["Wk"], np.float32).astype(ml_dtypes.bfloat16))
    Wq = np.ascontiguousarray(np.asarray(inputs["Wq"], np.float32).astype(ml_dtypes.bfloat16))
    b1 = np.asarray(inputs["b1"], np.float64)
    w2 = np.asarray(inputs["w2"], np.float64)
    cf32, cb16, cfp16 = _host_tables(b1, w2, Wk)

    in_maps = []
    for c in range(NCORES):
        sl = slice(c * BL, (c + 1) * BL)
        in_maps.append({
            "queries": queries[sl], "keys": keys[sl], "values": values[sl],
            "Wq": Wq, "Wk": Wk, "cf32": cf32, "cb16": cb16, "cfp16": cfp16,
        })
    return in_maps


def _run(inputs, trace=False, trace_kwargs=None):
    nc = _get_nc()
    in_maps = _make_in_maps(inputs)
    kwargs = {}
    if trace:
        kwargs = dict(trace=True, trace_cores=[0], trace_kwargs=trace_kwargs or {})
    res = run_bass_kernel_spmd(nc, in_maps, core_ids=list(range(NCORES)), **kwargs)
    out = np.concatenate([res.results[c]["out"] for c in range(NCORES)], axis=0)
    return out, res


def kernel(**inputs) -> np.ndarray:
    out, _ = _run(inputs, trace=False)
    return out
